# revision 43
# baseline (speedup 1.0000x reference)
"""Trainium2 Bass kernel for a cross-attention module.

Computes, per batch b (all shapes hardcoded; B=8, L=2048, D=H=1024):
    mapped_a = input_a @ Wa.T + ba            [L, H]
    mapped_b = input_b @ Wb.T + bb            [L, H]
    S        = mapped_a @ mapped_b.T          [L, L]
    attn_a   = softmax(S, axis=-1)            (rows)
    attn_b   = softmax(S.T, axis=-1)          (cols of S)
    out_a    = attn_b @ input_b               [L, D]
    out_b    = attn_a.T @ input_a             [L, D]
    out_ab   = out_a @ Wab.T + bab + out_b @ Wba.T

Sharding: data-parallel over batch across the 8 NeuronCores (one batch per
core); weights replicated.  Per-core pipeline:

  P1: stream input/weight fp32 tiles in, PE-transpose (via identity matmul)
      into fp16 [d, l] / [d, h] operands, cast bf16 naturals, and run the
      projection matmuls (fp16) chunk-by-chunk -> mapped^T [h, l].
  P2: scores (fp16) + ACT exp(S - 96) -> E (bf16); rowsum rides the
      activation accum_out; colsum accumulates via M=1 ones-matmuls over E.
  P3: value matmuls (bf16) -> out_a, out_b; colsum normalization applied on
      PSUM eviction; out^T produced by PE-transposing the evicted tiles.
  P4: output projection (bf16) with bias via a rank-1 ones matmul.

The softmax subtracts one global constant (C=96): a scalar shift is valid for
both softmax directions simultaneously, so a single E serves both.  exp is
evaluated in fp32 by ACT and stored bf16 (bf16 covers e^-180..e^0; fp16
would flush entire weak columns to zero).  fp16 is used for the
projection/score operands (same PE throughput as bf16, 8 more mantissa
bits); E and everything downstream is bf16.  All matmuls accumulate fp32.

SBUF slot groups are reused across phases via tile tags (Tile pools release
strictly LIFO): big1 = mapped^T then out_a^T/out_b^T; big2 = E then
Wab^T/Wba^T.
"""

import sys
from contextlib import ExitStack

import numpy as np

sys.path.insert(0, "/opt/trn_rl_repo")

import concourse.bacc as bacc
import concourse.bass as bass
import concourse.mybir as mybir
import concourse.tile as tile
from concourse.bass_utils import run_bass_kernel_spmd
from concourse.masks import make_identity

B, L, D, H = 8, 2048, 1024, 1024
P = 128
LT = L // P  # 16 row tiles
DT = D // P  # 8 contraction tiles over d
HT = H // P  # 8 tiles over h
NCH = 512    # free-dim chunk = one fp32 PSUM bank
NJ = L // NCH  # 4 l/m chunks
C_SHIFT = 96.0

F32 = mybir.dt.float32
F16 = mybir.dt.float16
BF16 = mybir.dt.bfloat16
AF = mybir.ActivationFunctionType
AX = mybir.AxisListType
ts = bass.ts


def build_cross_attention(nc, tc):
    inp = {
        "a": nc.dram_tensor("input_a", [L, D], F32, kind="ExternalInput").ap(),
        "b": nc.dram_tensor("input_b", [L, D], F32, kind="ExternalInput").ap(),
    }
    Wa = nc.dram_tensor("Wa", [H, D], F32, kind="ExternalInput").ap()
    ba = nc.dram_tensor("ba", [H], F32, kind="ExternalInput").ap()
    Wb = nc.dram_tensor("Wb", [H, D], F32, kind="ExternalInput").ap()
    bb = nc.dram_tensor("bb", [H], F32, kind="ExternalInput").ap()
    Wab = nc.dram_tensor("Wab", [H, H], F32, kind="ExternalInput").ap()
    bab = nc.dram_tensor("bab", [H], F32, kind="ExternalInput").ap()
    Wba = nc.dram_tensor("Wba", [H, H], F32, kind="ExternalInput").ap()
    out_a = nc.dram_tensor("out_a", [L, D], F32, kind="ExternalOutput").ap()
    out_b = nc.dram_tensor("out_b", [L, D], F32, kind="ExternalOutput").ap()
    out_ab = nc.dram_tensor("out_ab", [L, H], F32, kind="ExternalOutput").ap()

    ctx = ExitStack()
    with ctx:
        dram = ctx.enter_context(tc.tile_pool(name="dram_scratch", bufs=1, space="DRAM"))
        wab_bfs = dram.tile([H, H], BF16, name="wab_bfs", tag="wab_bfs")
        wba_bfs = dram.tile([H, H], BF16, name="wba_bfs", tag="wba_bfs")

        const = ctx.enter_context(tc.tile_pool(name="const", bufs=1))
        id_f32 = const.tile([P, P], F32, name="id_f32", tag="id_f32")
        make_identity(nc, id_f32[:])
        id_bf = const.tile([P, P], BF16, name="id_bf", tag="id_bf")
        make_identity(nc, id_bf[:])
        id_f16 = const.tile([P, P], F16, name="id_f16", tag="id_f16")
        make_identity(nc, id_f16[:])
        ones_mat = const.tile([P, P], BF16, name="ones_mat", tag="ones_mat")
        nc.vector.memset(ones_mat[:], 1.0)
        ones_row = const.tile([1, P], BF16, name="ones_row", tag="ones_row")
        nc.vector.memset(ones_row[:], 1.0)
        neg_c = const.tile([P, 1], F32, name="neg_c", tag="neg_c")
        nc.vector.memset(neg_c[:], -C_SHIFT)
        ba_col = const.tile([P, HT], F32, name="ba_col", tag="ba_col")
        nc.sync.dma_start(ba_col[:], ba.rearrange("(t p) -> p t", p=P))
        bb_col = const.tile([P, HT], F32, name="bb_col", tag="bb_col")
        nc.sync.dma_start(bb_col[:], bb.rearrange("(t p) -> p t", p=P))
        bab_row = const.tile([1, H], BF16, name="bab_row", tag="bab_row")

        stats = ctx.enter_context(tc.tile_pool(name="stats", bufs=1))
        recip_rs = stats.tile([P, LT], F32, name="recip_rs", tag="recip_rs")
        rc_all = stats.tile([P, LT], F32, name="rc_all", tag="rc_all")

        # big1 slots: mapped^T (P1-P2) then oaT/obT (P3-P4)
        big1 = ctx.enter_context(tc.tile_pool(name="big1", bufs=1))
        map_T = {
            "a": [big1.tile([P, L], F16, name=f"map_aT{k}", tag=f"s{k}") for k in range(HT)],
            "b": [big1.tile([P, L], F16, name=f"map_bT{k}", tag=f"s{HT + k}") for k in range(HT)],
        }

        # ---- P1: load + PE-transpose weights/inputs, projections ----
        with (
            tc.tile_pool(name="cast", bufs=6) as cast_pool,
            tc.tile_pool(name="w16T", bufs=1) as w16p,
            tc.tile_pool(name="chunks", bufs=2) as chpool,
            tc.tile_pool(name="psum_tp", bufs=3, space="PSUM") as tp1,
            tc.tile_pool(name="psum1", bufs=5, space="PSUM") as psum1,
        ):
            # waT[x] is [d(part), dtile, h]: slice [:, k, :] is the [d, h]
            # weight block for contraction tile k.
            waT = {
                "a": w16p.tile([P, DT, H], F16, name="waTa", tag="waTa"),
                "b": w16p.tile([P, DT, H], F16, name="waTb", tag="waTb"),
            }
            bab_f32 = cast_pool.tile([1, H], F32, name="bab_f32", tag="bab32", bufs=1)
            nc.sync.dma_start(bab_f32[:], bab[None, :])
            nc.vector.tensor_copy(bab_row[:], bab_f32[:])

            def w_tile(x, Wsrc, i):
                # one weight row-tile: load fp32, cast f16, PE-transpose the
                # eight 128x128 blocks into waT[x][:, :, i*128:...]
                w32 = cast_pool.tile([P, D], F32, name="w32", tag="c32")
                (nc.sync, nc.scalar, nc.gpsimd)[i % 3].dma_start(w32[:], Wsrc[ts(i, P), :])
                w16 = cast_pool.tile([P, D], F16, name="w16", tag="c16", bufs=4)
                nc.vector.tensor_copy(w16[:], w32[:])
                for kk in range(0, DT, 4):
                    ptp = tp1.tile([P, 4, P], F16, name="ptp", tag="tp")
                    for q in range(4):
                        nc.tensor.transpose(ptp[:, q, :], w16[:, ts(kk + q, P)], id_f16[:])
                    nc.scalar.copy(waT[x][:, kk:kk + 4, ts(i, P)], ptp[:])

            def in_chunk(x, j, bcol):
                # one input l-chunk: 4 row tiles -> bf16 naturals + f16
                # transposed chunk [d(part), dtile, l-chunk]; then projections.
                ch = chpool.tile([P, DT, NCH], F16, name=f"ch{x}", tag=f"ch{x}")
                for q in range(4):
                    lt = 4 * j + q
                    c32 = cast_pool.tile([P, D], F32, name="c32", tag="c32")
                    (nc.sync, nc.scalar, nc.gpsimd)[(4 * j + q) % 3].dma_start(c32[:], inp[x][ts(lt, P), :])
                    c16 = cast_pool.tile([P, D], F16, name="c16", tag="c16", bufs=4)
                    nc.vector.tensor_copy(c16[:], c32[:])
                    for kk in range(0, DT, 4):
                        ptp = tp1.tile([P, 4, P], F16, name="ptpi", tag="tp")
                        for r in range(4):
                            nc.tensor.transpose(ptp[:, r, :], c16[:, ts(kk + r, P)], id_f16[:])
                        nc.vector.tensor_copy(ch[:, kk:kk + 4, ts(q, P)], ptp[:])
                for i in range(HT):
                    ps = psum1.tile([P, NCH], F32, name="ps1", tag="ps1")
                    for k in range(DT):
                        nc.tensor.matmul(ps[:], waT[x][:, k, ts(i, P)], ch[:, k, :],
                                         start=(k == 0), stop=(k == DT - 1))
                    nc.scalar.activation(map_T[x][i][:, ts(j, NCH)], ps[:], AF.Identity,
                                         bias=bcol[:, i:i + 1])

            for i in range(HT):
                w_tile("b", Wb, i)
            for j in range(NJ):
                # trickle Wa tiles between b-chunks so their DMA and PE work
                # overlap the b-projection stream
                w_tile("a", Wa, 2 * j)
                w_tile("a", Wa, 2 * j + 1)
                in_chunk("b", j, bb_col)
            for j in range(NJ):
                in_chunk("a", j, ba_col)

        # natural-layout bf16 inputs, written during P2 (input re-read: the
        # DMA system is idle there), read by P3.  input_a rows are scaled by
        # 1/rowsum as part of the cast (tensor_scalar does cast+scale).
        natpool = ctx.enter_context(tc.tile_pool(name="natp", bufs=1))
        natp = {
            x: [natpool.tile([P, D], BF16, name=f"nat{x}{k}", tag=f"nat{x}{k}")
                for k in range(LT)]
            for x in ("a", "b")
        }

        # big2 slots: E (P2-P3) then WabT/WbaT (P4)
        big2 = ctx.enter_context(tc.tile_pool(name="big2", bufs=1))
        E = [big2.tile([P, L], BF16, name=f"E{i}", tag=f"e{i}") for i in range(LT)]

        # ---- P2: scores + exp -> E; rowsum via accum_out; colsum matmuls ----
        with (
            tc.tile_pool(name="psum2", bufs=4, space="PSUM") as psum2,
            tc.tile_pool(name="psum_cs", bufs=1, space="PSUM") as psum_cs,
            tc.tile_pool(name="rsparts", bufs=2) as rsp_pool,
            tc.tile_pool(name="wabst", bufs=2) as wabst_pool,
            tc.tile_pool(name="cs_sb", bufs=1) as cs_pool,
        ):
            # colsum accumulators: ones-matrix matmul broadcasts each chunk's
            # column sums across all 128 partitions.
            pcs = [psum_cs.tile([P, NCH], F32, name=f"pcs{c}", tag=f"pcs{c}") for c in range(NJ)]
            wab_jobs = [(Wab, wab_bfs, i) for i in range(HT)] + [(Wba, wba_bfs, i) for i in range(HT)]
            for i in range(LT):
                rsp = rsp_pool.tile([P, NJ], F32, name="rsp", tag="rsp")
                for j in range(NJ):
                    ps = psum2.tile([P, NCH], F32, name="ps2", tag="ps2")
                    for k in range(HT):
                        nc.tensor.matmul(ps[:], map_T["a"][k][:, ts(i, P)], map_T["b"][k][:, ts(j, NCH)],
                                         start=(k == 0), stop=(k == HT - 1))
                    nc.scalar.activation(E[i][:, ts(j, NCH)], ps[:], AF.Exp,
                                         bias=neg_c[:], accum_out=rsp[:, j:j + 1])
                # stage one Wab/Wba row-tile per l-tile: the scalar queue's
                # FIFO order paces this traffic behind the exp stream, so it
                # never competes with P1's latency-critical loads.
                if i < len(wab_jobs):
                    Wsrc, dst, wi = wab_jobs[i]
                    w32b = wabst_pool.tile([P, D], F32, name="w32b", tag="c32b")
                    nc.scalar.dma_start(w32b[:], Wsrc[ts(wi, P), :])
                    wbf = wabst_pool.tile([P, D], BF16, name="wbf", tag="cbf", bufs=1)
                    nc.vector.tensor_copy(wbf[:], w32b[:])
                    nc.scalar.dma_start(dst[ts(wi, P), :], wbf[:])
                for j in range(NJ):
                    nc.tensor.matmul(pcs[j][:], ones_mat[:], E[i][:, ts(j, NCH)],
                                     start=(i == 0), stop=(i == LT - 1))
                rs1 = rsp_pool.tile([P, 1], F32, name="rs1", tag="rs1")
                nc.vector.reduce_sum(rs1[:], rsp[:], axis=AX.X)
                nc.vector.reciprocal(recip_rs[:, i:i + 1], rs1[:])
                # re-read this l-tile of both inputs and produce the bf16
                # naturals: input_a scaled by 1/rowsum in the same DVE op.
                ra = wabst_pool.tile([P, D], F32, name="ra", tag="c32b")
                nc.scalar.dma_start(ra[:], inp["a"][ts(i, P), :])
                nc.vector.tensor_scalar_mul(natp["a"][i][:], ra[:], recip_rs[:, i:i + 1])
                rb = wabst_pool.tile([P, D], F32, name="rb", tag="c32b")
                nc.scalar.dma_start(rb[:], inp["b"][ts(i, P), :])
                nc.vector.tensor_copy(natp["b"][i][:], rb[:])

            # colsum -> partition-indexed [128, 16]: copy broadcast rows to
            # SBUF, PE-transpose each 128-block, take one column per block.
            csg = cs_pool.tile([P, LT], F32, name="csg", tag="csg")
            for j in range(NJ):
                # alternate engines so the four chunk chains overlap: this
                # chain gates the P2->P3 PSUM pool handoff.
                csf = wabst_pool.tile([P, NCH], F32, name="csf", tag="c32b")
                if j % 2 == 0:
                    nc.vector.tensor_copy(csf[:], pcs[j][:])
                else:
                    nc.scalar.copy(csf[:], pcs[j][:])
                cst = psum2.tile([P, NCH], F32, name="cst", tag="ps2")
                for q in range(4):
                    nc.tensor.transpose(cst[:, ts(q, P)], csf[:, ts(q, P)], id_f32[:])
                for q in range(4):
                    if j % 2 == 0:
                        nc.vector.tensor_copy(csg[:, 4 * j + q:4 * j + q + 1], cst[:, q * P:q * P + 1])
                    else:
                        nc.scalar.copy(csg[:, 4 * j + q:4 * j + q + 1], cst[:, q * P:q * P + 1])
            nc.vector.reciprocal(rc_all[:], csg[:])

        # ---- P3: out_a / out_b value matmuls; evict + PE-transpose ----
        oT = {
            "a": [big1.tile([P, L], BF16, name=f"oaT{k}", tag=f"s{k}") for k in range(DT)],
            "b": [big1.tile([P, L], BF16, name=f"obT{k}", tag=f"s{DT + k}") for k in range(DT)],
        }
        with (
            tc.tile_pool(name="psum3", bufs=5, space="PSUM") as psum3,
            tc.tile_pool(name="psum_tp3", bufs=3, space="PSUM") as tp3,
            tc.tile_pool(name="stage3", bufs=3) as stage3,
            tc.tile_pool(name="stage3b", bufs=3) as stage3b,
        ):
            for i in range(LT):
                po = {x: [psum3.tile([P, NCH], F32, name=f"po{x}{c}", tag="ps3") for c in range(2)]
                      for x in ("a", "b")}
                for k in range(LT):
                    lhs = E[k][:, ts(i, P)]
                    st, sp = (k == 0), (k == LT - 1)
                    nc.tensor.matmul(po["a"][0][:], lhs, natp["b"][k][:, 0:NCH], start=st, stop=sp)
                    nc.tensor.matmul(po["a"][1][:], lhs, natp["b"][k][:, NCH:D], start=st, stop=sp)
                    nc.tensor.matmul(po["b"][0][:], lhs, natp["a"][k][:, 0:NCH], start=st, stop=sp)
                    nc.tensor.matmul(po["b"][1][:], lhs, natp["a"][k][:, NCH:D], start=st, stop=sp)
                rows = ts(i, P)
                for x, dst in (("a", out_a), ("b", out_b)):
                    for c in range(2):
                        cols = ts(c, NCH)
                        of = stage3.tile([P, NCH], F32, name="of", tag="of")
                        obf = stage3b.tile([P, NCH], BF16, name="obf", tag="obf")
                        if x == "a":
                            nc.scalar.mul(of[:], po[x][c][:], rc_all[:, i:i + 1])
                            nc.vector.tensor_scalar_mul(obf[:], po[x][c][:], rc_all[:, i:i + 1])
                        else:
                            nc.scalar.copy(of[:], po[x][c][:])
                            nc.vector.tensor_copy(obf[:], po[x][c][:])
                        nc.sync.dma_start(dst[rows, cols], of[:])
                        ptp = tp3.tile([P, 4 * P], BF16, name="ptp3", tag="tp3")
                        for q in range(4):
                            nc.tensor.transpose(ptp[:, ts(q, P)], obf[:, ts(q, P)], id_bf[:])
                        for q in range(4):
                            nc.vector.tensor_copy(oT[x][4 * c + q][:, ts(i, P)], ptp[:, ts(q, P)])

        # ---- P4: out_ab = out_a @ Wab.T + bab + out_b @ Wba.T ----
        with (
            tc.tile_pool(name="psum4", bufs=6, space="PSUM") as psum4,
            tc.tile_pool(name="psum_tp4", bufs=2, space="PSUM") as tp4,
            tc.tile_pool(name="stage4", bufs=4) as stage4,
        ):
            wT2 = {
                "a": [big2.tile([P, H], BF16, name=f"wabT{k}", tag=f"e{k}") for k in range(HT)],
                "b": [big2.tile([P, H], BF16, name=f"wbaT{k}", tag=f"e{HT + k}") for k in range(HT)],
            }
            # Load the bf16 weight scratch natural-layout into the natp slots
            # (free after P3) on two parallel queues, then PE-transpose into
            # the e-slots: the PE is idle in this window anyway, and plain
            # loads beat a serial xbar chain.
            wnat = {
                x: [natpool.tile([P, D], BF16, name=f"wn{x}{i}", tag=f"nat{x}{i}")
                    for i in range(HT)]
                for x in ("a", "b")
            }
            for i in range(HT):
                nc.sync.dma_start(wnat["a"][i][:], wab_bfs[ts(i, P), :])
                nc.gpsimd.dma_start(wnat["b"][i][:], wba_bfs[ts(i, P), :])
            for x in ("a", "b"):
                for k in range(DT):
                    for ii in range(0, HT, 4):
                        # transpose block k of four consecutive weight rows:
                        # the four outputs form one contiguous [128, 512]
                        # slice of wT2[x][k], evicted with a single copy.
                        ptp4 = tp4.tile([P, 4, P], BF16, name="ptp4", tag="tp4")
                        for q in range(4):
                            nc.tensor.transpose(ptp4[:, q, :], wnat[x][ii + q][:, ts(k, P)], id_bf[:])
                        eng = nc.vector if k % 2 == 0 else nc.scalar
                        if k % 2 == 0:
                            nc.vector.tensor_copy(wT2[x][k][:, ii * P:(ii + 4) * P], ptp4[:])
                        else:
                            nc.scalar.copy(wT2[x][k][:, ii * P:(ii + 4) * P], ptp4[:])

            for i in range(LT):
                pab = [psum4.tile([P, NCH], F32, name=f"pab{c}", tag="ps4") for c in range(2)]
                for x in ("a", "b"):
                    for k in range(HT):
                        lhs = oT[x][k][:, ts(i, P)]
                        st = (x == "a" and k == 0)
                        nc.tensor.matmul(pab[0][:], lhs, wT2[x][k][:, 0:NCH], start=st, stop=False)
                        nc.tensor.matmul(pab[1][:], lhs, wT2[x][k][:, NCH:H], start=st, stop=False)
                nc.tensor.matmul(pab[0][:], ones_row[:], bab_row[:, 0:NCH], start=False, stop=True)
                nc.tensor.matmul(pab[1][:], ones_row[:], bab_row[:, NCH:H], start=False, stop=True)
                rows = ts(i, P)
                for c in range(2):
                    abf = stage4.tile([P, NCH], F32, name="abf", tag="abf")
                    nc.scalar.copy(abf[:], pab[c][:])
                    nc.sync.dma_start(out_ab[rows, ts(c, NCH)], abf[:])


def build_nc(debug=False):
    nc = bacc.Bacc("TRN2", target_bir_lowering=False, debug=debug)
    with tile.TileContext(nc) as tc:
        build_cross_attention(nc, tc)
    nc.compile()
    return nc


_COMPILED_NC = None


def kernel(**inputs):
    global _COMPILED_NC
    if _COMPILED_NC is None:
        _COMPILED_NC = build_nc(debug=False)
    nc = _COMPILED_NC

    inputs = {k: np.ascontiguousarray(np.asarray(v)) for k, v in inputs.items()}
    in_maps = []
    for i in range(B):
        in_maps.append({
            "input_a": inputs["input_a"][i],
            "input_b": inputs["input_b"][i],
            "Wa": inputs["Wa"], "ba": inputs["ba"],
            "Wb": inputs["Wb"], "bb": inputs["bb"],
            "Wab": inputs["Wab"], "bab": inputs["bab"],
            "Wba": inputs["Wba"],
        })
    res = run_bass_kernel_spmd(nc, in_maps, core_ids=list(range(B)))
    out_a = np.stack([res.results[i]["out_a"] for i in range(B)]).astype(np.float32)
    out_b = np.stack([res.results[i]["out_b"] for i in range(B)]).astype(np.float32)
    out_ab = np.stack([res.results[i]["out_ab"] for i in range(B)]).astype(np.float32)
    return out_a, out_b, out_ab
